# revision 74
# baseline (speedup 1.0000x reference)
"""Sparse attention (per-query top-K) Trainium2 kernel, 8-core tensor-parallel.

Strategy (heads sharded 2-per-core, dense-score formulation):
  - Host folds idx/valid/geo_bias into per-(s,q) merged bias factors
    E[s,q] = sum_{j: idx[q,j]==s} exp(geo_bias[h,q,j]), stored as causal
    fp16 tiles.  This turns the per-query gather/softmax into dense math:
        A^T = E^T * exp(S^T - C),   S^T = K @ Q^T (feature-major)
        out^T = [V | 1]^T @ A^T     (row 64 = softmax denominator)
  - Single software-pipelined main loop: the projection matmuls for
    query-tile t+1 (bf16, x pre-transposed on host) are interleaved into
    the attention chunk loop of tile t so the PE fills the slack while
    ACT (the exp bottleneck) streams.  The two heads' 64-contraction
    score matmuls run concurrently in row-halves of the PE array; exp on
    ACT covers both heads per instruction; the E-multiply runs on DVE
    once per 4 chunks (fp16 2x mode); AV on PE in fp16.
  - AllToAll reshards the (unnormalized) head outputs + denominators so
    each core owns 512 query rows with all 16 heads, normalizes, and
    computes its o_proj row-slice with the full Wo in bf16.  Host
    concatenates.
"""

import sys

sys.path.insert(0, "/opt/trn_rl_repo")

import numpy as np
import ml_dtypes

from concourse import bacc, mybir, tile
from concourse.bass_utils import run_bass_kernel_spmd
from concourse.masks import make_identity

F32 = mybir.dt.float32
F16 = mybir.dt.float16
BF16 = mybir.dt.bfloat16

S = 4096
H = 1024
NH = 16
KSEL = 32
HD = 64
NC = 8
HPC = NH // NC  # 2 heads per core
QT = 512
NQT = S // QT
SC = 128
CSHIFT = 2.0
SLAB = 4  # s-chunks per E-tile DMA slab
QUAD = 2  # chunks per DVE multiply
AVLAG = 2  # quads the AV matmul trails the exp/multiply pipeline

TILE_LIST = [(t, c) for t in range(NQT) for c in range(4 * (t + 1))]
N_TILES = len(TILE_LIST)  # 144
TILE_IDX = {tc: n for n, tc in enumerate(TILE_LIST)}


def _build_program(phases=3, n_cores_build=NC):
    nc = bacc.Bacc(
        "TRN2", target_bir_lowering=False, debug=False, num_devices=n_cores_build
    )

    # host-pretransposed x: [st, p, hc, 512] (feature-major, bf16)
    xT_in = nc.dram_tensor("xT", [NQT, 128, 8, QT], BF16, kind="ExternalInput").ap()
    wq_in = nc.dram_tensor("wq", [128, 8, 128], BF16, kind="ExternalInput").ap()
    wk_in = nc.dram_tensor("wk", [128, 8, 128], BF16, kind="ExternalInput").ap()
    wv_in = nc.dram_tensor("wv", [128, 8, 128], BF16, kind="ExternalInput").ap()
    wo_in = nc.dram_tensor("wo", [128, 8, H], BF16, kind="ExternalInput").ap()
    bo_in = nc.dram_tensor("bo_rep", [128, H], F32, kind="ExternalInput").ap()
    e_in = nc.dram_tensor(
        "e_pack", [N_TILES, SC, HPC, QT], F16, kind="ExternalInput"
    ).ap()
    sel_in = nc.dram_tensor("sel16", [NH, H], F32, kind="ExternalInput").ap()
    wown_in = nc.dram_tensor(
        "wo_own", [HD, HPC, H], BF16, kind="ExternalInput"
    ).ap()
    sel7_in = nc.dram_tensor("sel7", [HPC, 128], F32, kind="ExternalInput").ap()
    y_out = nc.dram_tensor("y_part", [448, H], F16, kind="ExternalOutput").ap()
    # tile 7 is not resharded: each core emits its 2-head partial of the
    # full 512 rows; the host sums the 8 partials
    y7_out = nc.dram_tensor("y7_part", [QT, H], F16, kind="ExternalOutput").ap()

    with tile.TileContext(nc) as tc:
        with (
            tc.tile_pool(name="const", bufs=1) as constp,
            tc.tile_pool(name="persist", bufs=1) as persist,
            tc.tile_pool(name="dram", bufs=1, space="DRAM") as dram,
        ):
            ident = constp.tile([128, 128], F32, tag="ident")
            nbias = constp.tile([128, 1], F32, tag="nbias")

            wq_sb = constp.tile([128, 8, 128], BF16, tag="wq")
            wk_sb = constp.tile([128, 8, 128], BF16, tag="wk")
            wv_sb = constp.tile([128, 8, 128], BF16, tag="wv")
            # wo/bo/sel are loaded later (mid phase 2) to keep startup lean
            wo_sb = constp.tile([128, 8, H], BF16, tag="wo")
            bo_sb = constp.tile([128, H], F32, tag="bo")
            sel_sb = constp.tile([NH, H], F32, tag="sel")
            wown_sb = constp.tile([HD, HPC, H], BF16, tag="wown")
            # sel7[h, col] = 1 where col's head-dim block belongs to head h
            sel7 = constp.tile([HPC, 128], F32, tag="sel7")

            qT_sb = persist.tile([128, NQT, QT], BF16, tag="qT")
            kT_sb = persist.tile([128, NQT, QT], BF16, tag="kT")
            # wave-0/1 phase-3 inputs live in the persistent pool so their
            # loads + compute can be issued from inside the main loop
            den0_sb = persist.tile([NH, 384], F16, tag="den0")
            rden0_sb = persist.tile([NH, 384], F32, tag="rden0")
            oT0_sb = persist.tile([128, 8, 384], F16, tag="oT0")
            den1_sb = persist.tile([NH, 64], F16, tag="den1")
            rden1_sb = persist.tile([NH, 64], F32, tag="rden1")
            oT1_sb = persist.tile([128, 8, 64], F16, tag="oT1")
            v_sb = [
                persist.tile([128, S // SC, HD + 1], F16, tag=f"v{h}", name=f"v{h}")
                for h in range(HPC)
            ]


            # two AllToAll waves: wave 0 = q-tiles 0..5 (384 q/peer),
            # wave 1 = q-tile 6 (64 q/peer) — both resharded early enough
            # that their phase-3 interleaves into tile 7's chunk loop.
            # Tile 7 itself is never resharded: each core computes its
            # 2-head o_proj partial locally and the host sums them.
            WTILES = [list(range(0, 6)), [6]]
            WQ = [len(ws) * QT // NC for ws in WTILES]  # 384, 64
            NW = len(WTILES)
            WT0 = [ws[0] for ws in WTILES]  # first tile of each wave
            a2a_in = [
                dram.tile(
                    [NC, HPC * (HD + 1), WQ[w]], F16, tag=f"ain{w}", name=f"ain{w}"
                )
                for w in range(NW)
            ]
            a2a_out = [
                dram.tile(
                    [NC, HPC * (HD + 1), WQ[w]], F16, tag=f"aout{w}", name=f"aout{w}"
                )
                for w in range(NW)
            ]

            main_pools = (
                tc.tile_pool(name="xT", bufs=3),
                tc.tile_pool(name="vtmp", bufs=2),
                tc.tile_pool(name="zap", bufs=3),
                tc.tile_pool(name="ap", bufs=AVLAG + 2),
                tc.tile_pool(name="ep", bufs=7),
                tc.tile_pool(name="otp", bufs=2),
                tc.tile_pool(name="p3s", bufs=1),
                tc.tile_pool(name="p2s", bufs=2, space="PSUM"),
                tc.tile_pool(name="p2o", bufs=1, space="PSUM"),
                tc.tile_pool(name="p1ps", bufs=1, space="PSUM"),
            )
            xTp, vtmpp, zap, apool, epool, otp, p3s, p2s, p2o, p1ps = (
                p.__enter__() for p in main_pools
            )
            xT_tiles = {}

            # Queue discipline: E-slabs + xT alternate sync/gpsimd (the
            # two bulk queues); staging + y outputs ride the scalar queue
            # (their waits are always short, so the exp stream isn't
            # blocked); phase-3 reshard loads ride gpsimd where the only
            # long waits (on collectives) have dedicated slack.
            dma_no = [0]
            slab_no = [0]

            def big_dma(dst, src):
                if dma_no[0] % 2 == 0:
                    nc.sync.dma_start(dst, src)
                else:
                    nc.gpsimd.dma_start(dst, src)
                dma_no[0] += 1

            def load_xT(st, split=False):
                xt = xTp.tile([128, 8, QT], BF16, tag="xT", name=f"xT{st}")
                xT_tiles[st] = xt
                if split:  # per-hc chunks so the first matmul starts ASAP
                    for hc in range(8):
                        nc.sync.dma_start(xt[:, hc, :], xT_in[st, :, hc, :])
                    dma_no[0] += 1
                else:
                    big_dma(xt[:], xT_in[st])

            # ---- projection work for one s-tile, as two piece lists ----
            def proj_pieces(st):
                pieces = []
                state = {}

                def mk_mm(w_sb, crange, tag, fin):
                    def go():
                        if tag not in state:
                            state[tag] = p1ps.tile([128, QT], F32, tag="proj", name=f"proj_{tag}")
                        ps = state[tag]
                        for c in crange:
                            nc.tensor.matmul(
                                ps[:], w_sb[:, c, :], xT_tiles[st][:, c, :],
                                start=(c == 0), stop=(c == 7),
                            )
                        if fin is not None:
                            fin(ps)

                    return go

                def fin_q(ps):
                    nc.vector.tensor_copy(qT_sb[:, st, :], ps[:])

                def fin_k(ps):
                    nc.vector.tensor_copy(kT_sb[:, st, :], ps[:])

                def fin_v(ps):
                    vt = vtmpp.tile([128, QT], F32, tag="vt", name="vt")
                    state["vt"] = vt
                    nc.vector.tensor_copy(vt[:], ps[:])

                qk = [
                    mk_mm(wq_sb, range(0, 4), "q", None),
                    mk_mm(wq_sb, range(4, 8), "q", fin_q),
                    mk_mm(wk_sb, range(0, 4), "k", None),
                    mk_mm(wk_sb, range(4, 8), "k", fin_k),
                ]
                pieces.append(mk_mm(wv_sb, range(0, 4), "v", None))
                pieces.append(mk_mm(wv_sb, range(4, 8), "v", fin_v))

                def transpose_v():
                    vt = state["vt"]
                    ps_tv = p1ps.tile([128, QT], F32, tag="tp")
                    for i in range(4):
                        nc.tensor.transpose(
                            ps_tv[:, i * 128 : (i + 1) * 128],
                            vt[:, i * 128 : (i + 1) * 128],
                            ident[:],
                        )
                    ps_tv4 = ps_tv[:].rearrange("p (i h d) -> p i h d", i=4, h=HPC)
                    for h in range(HPC):
                        nc.vector.tensor_copy(
                            v_sb[h][:, st * 4 : (st + 1) * 4, 0:HD],
                            ps_tv4[:, :, h, :],
                        )

                pieces.append(transpose_v)
                return qk, pieces

            # ---- phase 3 (normalize + o_proj) for one wave, as pieces ----
            # y row ranges per wave in y_part: wave 0 -> 0:384,
            # wave 1 -> 384:448
            W0OFF = [0, 384]

            def p3_load(w, den_sb, oT_sb):
                # a2a_out[w] -> den/oT, on the gpsimd queue (gated on the
                # wave's collective; nothing else needs gpsimd then)
                for l in range(HPC):
                    nc.gpsimd.dma_start(
                        den_sb[l * 8 : (l + 1) * 8, :],
                        a2a_out[w][:, l * (HD + 1) + HD, :],
                    )
                for l in range(HPC):
                    nc.gpsimd.dma_start(
                        oT_sb[l * HD : (l + 1) * HD, :, :],
                        a2a_out[w][
                            :, l * (HD + 1) : l * (HD + 1) + HD, :
                        ].rearrange("c d q -> d c q"),
                    )

            def phase3_pieces(w, den_sb, rden_sb, oT_sb, sbpool, pspool, ypool):
                qw = WQ[w]
                pieces = []
                state = {}

                def recip():
                    nc.vector.reciprocal(rden_sb[:], den_sb[:])

                pieces.append(recip)

                def mk_norm(ci0):
                    def go():
                        if "on" not in state:
                            state["on"] = sbpool.tile(
                                [128, 8, qw], BF16, tag=f"on{w}", name=f"on{w}"
                            )
                        on_sb = state["on"]
                        for ci in range(ci0, ci0 + 2):
                            ps_b = pspool.tile([128, QT], F32, tag="proj", name="p3b")
                            nc.tensor.matmul(
                                ps_b[:, 0:qw],
                                sel_sb[:, ci * 128 : (ci + 1) * 128],
                                rden_sb[:],
                                start=True,
                                stop=True,
                            )
                            nc.vector.tensor_mul(
                                on_sb[:, ci, :], oT_sb[:, ci, :], ps_b[:, 0:qw]
                            )

                    return go

                for ci0 in range(0, 8, 2):
                    pieces.append(mk_norm(ci0))

                for qb0 in range(0, qw, 128):
                    bsz = min(128, qw - qb0)

                    def mk_oproj(qb0=qb0, bsz=bsz):
                        def go():
                            on_sb = state["on"]
                            y_sb = ypool.tile([128, H], F16, tag="y", name="p3y")
                            for nh2 in range(2):
                                ps_y = pspool.tile([128, QT], F32, tag="tp", name="p3yp")
                                for c in range(8):
                                    nc.tensor.matmul(
                                        ps_y[0:bsz, :],
                                        on_sb[:, c, qb0 : qb0 + bsz],
                                        wo_sb[:, c, nh2 * QT : (nh2 + 1) * QT],
                                        start=(c == 0),
                                        stop=(c == 7),
                                    )
                                nc.vector.tensor_add(
                                    y_sb[0:bsz, nh2 * QT : (nh2 + 1) * QT],
                                    ps_y[0:bsz, :],
                                    bo_sb[0:bsz, nh2 * QT : (nh2 + 1) * QT],
                                )
                            nc.scalar.dma_start(
                                y_out[
                                    W0OFF[w] + qb0 : W0OFF[w] + qb0 + bsz, :
                                ],
                                y_sb[0:bsz, :],
                            )

                        return go

                    pieces.append(mk_oproj())
                return pieces

            # ---- main software-pipelined loop ---------------------------
            # first-needed bytes on separate queues so nothing serializes:
            # xT0 streams on sync (feeds the first matmuls), weights + xT1
            # go on gpsimd, and tile 0/1's E-slabs land right behind xT0
            # on sync (they gate the first DVE multiply)
            xt0 = xTp.tile([128, 8, QT], BF16, tag="xT", name="xT0")
            xT_tiles[0] = xt0
            for hc in range(8):
                nc.sync.dma_start(xt0[:, hc, :], xT_in[0, :, hc, :])
            nc.gpsimd.dma_start(wq_sb[:], wq_in[:])
            nc.gpsimd.dma_start(wk_sb[:], wk_in[:])
            nc.gpsimd.dma_start(wv_sb[:], wv_in[:])
            xt1 = xTp.tile([128, 8, QT], BF16, tag="xT", name="xT1")
            xT_tiles[1] = xt1
            nc.gpsimd.dma_start(xt1[:], xT_in[1])
            # constants AFTER the preamble DMA triggers: these gpsimd ops
            # are slow and must not delay the weight/x loads
            make_identity(nc, ident[:])
            nc.gpsimd.memset(nbias[:], -CSHIFT)
            # prime the exp table-set while the first DMAs stream
            warm = constp.tile([1, 1], F32, tag="warm")
            nc.scalar.activation(
                warm[:], nbias[0:1, :], mybir.ActivationFunctionType.Exp
            )
            for h in range(HPC):
                # only the ones-column matters; cols 0:64 are overwritten
                nc.gpsimd.memset(v_sb[h][:, :, HD : HD + 1], 1.0)
            qk0, v0 = proj_pieces(0)
            for p in qk0:
                p()
            carry_v = v0  # v-projection of tile t runs inside tile t's loop
            qk_next, v_next = proj_pieces(1)

            for t in range(NQT):
                nchunks = 4 * (t + 1)
                nquads = nchunks // QUAD
                # stream this tile's E slabs (demand-gated by pool bufs)
                slabs = []
                for g0 in range(0, nchunks, SLAB):
                    gsz = min(SLAB, nchunks - g0)
                    e_slab = epool.tile(
                        [128, SLAB, HPC, QT], F16, tag="e", name="e_slab"
                    )
                    n0 = TILE_IDX[(t, g0)]
                    big_dma(
                        e_slab[:, 0:gsz, :, :],
                        e_in[n0 : n0 + gsz].rearrange("n p h q -> p n h q"),
                    )
                    slabs.append(e_slab)
                if t == 7:
                    # wave-0 reshard results (collective fired at t=5);
                    # emitted after the slab triggers so gpsimd's slabs
                    # aren't stuck behind the collective-gated wait
                    p3_load(0, den0_sb, oT0_sb)
                if t + 2 < NQT:
                    load_xT(t + 2)
                if t == 4:
                    nc.gpsimd.dma_start(wo_sb[:], wo_in[:])
                    nc.gpsimd.dma_start(bo_sb[:], bo_in[:])
                    nc.gpsimd.dma_start(sel_sb[:], sel_in[:])
                    nc.gpsimd.dma_start(wown_sb[:], wown_in[:])
                    nc.gpsimd.dma_start(sel7[:], sel7_in[:])

                # pieces to interleave: this tile's V projection, then the
                # next tile's Q/K projection; at t == 7 also the phase-3
                # work of waves 0/1 (their collectives fired at t = 5/6)
                pieces = carry_v + qk_next
                if t + 1 < NQT:
                    carry_v = v_next
                    if t + 2 < NQT:
                        qk_next, v_next = proj_pieces(t + 2)
                    else:
                        qk_next = []
                qt_list = None
                if t == 7:
                    # waves 0/1 phase 3 interleave into tile 7's second
                    # half (wave-0's collective is in flight until
                    # ~mid-tile, wave-1's a bit longer)
                    p3w0 = phase3_pieces(0, den0_sb, rden0_sb, oT0_sb, p3s, p1ps, otp)
                    p3w1 = phase3_pieces(1, den1_sb, rden1_sb, oT1_sb, p3s, p1ps, otp)
                    n0 = len(pieces)
                    pieces = (
                        pieces + p3w0 + [lambda: p3_load(1, den1_sb, oT1_sb)] + p3w1
                    )
                    n2 = n0 + len(p3w0) + 1
                    qt_list = []
                    for qd in range(16):
                        if qd < 3:
                            qt_list.append(min(n0, qd + 1))
                        elif qd < 11:
                            qt_list.append(n0 + -(-(qd - 2) * len(p3w0) // 8))
                        else:
                            qt_list.append(n2 + -(-(qd - 10) * len(p3w1) // 5))
                npc = [0]

                def emit_proj_piece(n=1):
                    for _ in range(n):
                        if npc[0] < len(pieces):
                            pieces[npc[0]]()
                            npc[0] += 1

                ps_o = [
                    p2o.tile([HD + 1, QT], F32, tag=f"po{h}", name=f"po{h}")
                    for h in range(HPC)
                ]

                def emit_st_exp(t_, c, z_view, qlo=0):
                    ps_s2 = p2s.tile([128, 2 * QT], F32, tag="ps2", name="ps2")
                    for h in range(HPC):
                        nc.tensor.matmul(
                            ps_s2[:, h * QT + qlo : (h + 1) * QT],
                            kT_sb[
                                h * HD : (h + 1) * HD,
                                c // 4,
                                (c % 4) * 128 : (c % 4 + 1) * 128,
                            ],
                            qT_sb[h * HD : (h + 1) * HD, t_, qlo:],
                            start=True,
                            stop=True,
                        )
                    nc.scalar.activation(
                        z_view[:, :, qlo:],
                        ps_s2[:].rearrange("p (h q) -> p h q", h=HPC)[:, :, qlo:],
                        mybir.ActivationFunctionType.Exp,
                        bias=nbias[:],
                    )

                def chunk_qlo(c):
                    # columns below the causal diagonal of this s-chunk
                    # have E == 0; skip them
                    return max(0, 128 * (c - (nchunks - 4)))

                def emit_av(pc0, pa, last):
                    for pc in range(QUAD):
                        c = pc0 + pc
                        qlo = chunk_qlo(c)
                        for h in range(HPC):
                            nc.tensor.matmul(
                                ps_o[h][:, qlo:],
                                v_sb[h][:, c, :],
                                pa[:, pc, h, qlo:],
                                start=(c == 0),
                                stop=(last and c == nchunks - 1),
                            )

                # AV matmuls lag AVLAG quads behind the mul so the serial
                # scores->exp->mul->AV chain spreads over several slots
                pending_av = []
                for qd in range(nquads):
                    c0 = qd * QUAD
                    e_slab = slabs[c0 // SLAB]
                    z_sb = zap.tile([128, QUAD, HPC, QT], F16, tag="z", name="z4")
                    a_sb = apool.tile([128, QUAD, HPC, QT], F16, tag="a", name="a4")
                    for pc in range(QUAD):
                        emit_st_exp(
                            t, c0 + pc, z_sb[:, pc, :, :], chunk_qlo(c0 + pc)
                        )
                    c_loc = c0 % SLAB
                    # trimming the pair multiply to the smaller offset is
                    # safe: E is zero below the diagonal, so the extra
                    # columns multiply to exactly 0
                    qlo = chunk_qlo(c0)
                    nc.vector.tensor_mul(
                        a_sb[:, :, :, qlo:],
                        z_sb[:, :, :, qlo:],
                        e_slab[:, c_loc : c_loc + QUAD, :, qlo:],
                    )
                    if qt_list is not None:
                        emit_proj_piece(qt_list[qd] - npc[0])
                    else:
                        emit_proj_piece(
                            -(-(len(pieces) - npc[0]) // max(1, nquads - qd))
                            if pieces
                            else 0
                        )
                    pending_av.append((c0, a_sb))
                    if len(pending_av) > AVLAG:
                        pc0, pa = pending_av.pop(0)
                        emit_av(pc0, pa, False)
                emit_proj_piece(len(pieces))
                while pending_av:
                    pc0, pa = pending_av.pop(0)
                    emit_av(pc0, pa, not pending_av)

                if t < 7:
                    ot_sb = otp.tile([HD + 1, HPC, QT], F16, tag="ot")
                    for h in range(HPC):
                        nc.vector.tensor_copy(ot_sb[:, h, :], ps_o[h][:])
                    # stage this tile's columns into the owning peers' slots
                    w = 0 if t < 6 else 1
                    wq0 = WQ[w]
                    gcol0 = (t - WT0[w]) * QT
                    for p in range(NC):
                        lo = max(gcol0, wq0 * p)
                        hi = min(gcol0 + QT, wq0 * p + wq0)
                        if lo < hi:
                            nc.gpsimd.dma_start(
                                a2a_in[w][p][
                                    :, lo - wq0 * p : hi - wq0 * p
                                ].rearrange("(h pp) q -> pp h q", h=HPC),
                                ot_sb[:, :, lo - gcol0 : hi - gcol0],
                            )
                    if phases >= 2.5 and t in (5, 6):
                        w = t - 5
                        nc.gpsimd.collective_compute(
                            "AllToAll",
                            mybir.AluOpType.bypass,
                            replica_groups=[list(range(NC))],
                            ins=[a2a_in[w].opt()],
                            outs=[a2a_out[w].opt()],
                        )
                else:
                    # tile 7 local epilogue: normalize own 2 heads and emit
                    # this core's o_proj partial; the host sums the 8
                    # partials (no collective on the critical tail)
                    ot_sb = otp.tile([HD + 1, HPC, QT], F16, tag="ot")
                    for h in range(HPC):
                        nc.vector.tensor_copy(ot_sb[:, h, :], ps_o[h][:])
                    den7 = p3s.tile([HPC, QT], F16, tag="den7")
                    for h in range(HPC):
                        # partition shift (64 -> h) must go through DMA
                        nc.gpsimd.dma_start(
                            den7[h : h + 1, :], ot_sb[HD : HD + 1, h, :]
                        )
                    rden7 = p3s.tile([HPC, QT], F32, tag="rden7")
                    nc.vector.reciprocal(rden7[:], den7[:])
                    on7 = [
                        p3s.tile([HD, QT], BF16, tag=f"on7_{h}", name=f"on7_{h}")
                        for h in range(HPC)
                    ]
                    for h in range(HPC):
                        ps_b7 = p1ps.tile(
                            [128, QT], F32, tag="proj", name=f"bc7_{h}"
                        )
                        nc.tensor.matmul(
                            ps_b7[0:HD, :],
                            sel7[:, h * HD : (h + 1) * HD],
                            rden7[:],
                            start=True,
                            stop=True,
                        )
                        nc.vector.tensor_mul(
                            on7[h][:], ot_sb[0:HD, h, :], ps_b7[0:HD, :]
                        )
                    for qb in range(4):
                        ps_y7 = p2s.tile(
                            [128, 2 * QT], F32, tag="ps2", name="y7ps"
                        )
                        for nh2 in range(2):
                            for h in range(HPC):
                                nc.tensor.matmul(
                                    ps_y7[:, nh2 * QT : (nh2 + 1) * QT],
                                    on7[h][:, qb * 128 : (qb + 1) * 128],
                                    wown_sb[:, h, nh2 * QT : (nh2 + 1) * QT],
                                    start=(h == 0),
                                    stop=(h == HPC - 1),
                                )
                        y7_sb = otp.tile([128, H], F16, tag="y7", name="y7")
                        nc.vector.tensor_copy(y7_sb[:], ps_y7[:])
                        nc.scalar.dma_start(
                            y7_out[qb * 128 : (qb + 1) * 128, :], y7_sb[:]
                        )

            for p in reversed(main_pools):
                p.__exit__(None, None, None)

            if phases < 3:
                with tc.tile_pool(name="dbg", bufs=1) as dbgp:
                    dbg = dbgp.tile([128, 2, QT], F32, tag="dbg")
                    nc.vector.tensor_copy(dbg[:], qT_sb[:, 0:2, :].bitcast(F32))
                    nc.sync.dma_start(
                        y_out[0:128, :], dbg[:].rearrange("p a b -> p (a b)")
                    )

    nc.compile()
    return nc


_PROGRAM_CACHE = {}


def _get_program():
    if "nc" not in _PROGRAM_CACHE:
        _PROGRAM_CACHE["nc"] = _build_program()
    return _PROGRAM_CACHE["nc"]


def _host_prep(x, idx, valid, geo_bias, Wq, Wk, Wv, Wo, bo):
    x2 = np.ascontiguousarray(np.asarray(x, dtype=np.float32).reshape(S, H))
    idx = np.asarray(idx).astype(np.int64)
    valid = np.asarray(valid).astype(bool)
    geo = np.asarray(geo_bias, dtype=np.float32)
    Wq = np.asarray(Wq, dtype=np.float32)
    Wk = np.asarray(Wk, dtype=np.float32)
    Wv = np.asarray(Wv, dtype=np.float32)
    Wo = np.asarray(Wo, dtype=np.float32)
    bo = np.asarray(bo, dtype=np.float32)

    qpos = np.arange(S, dtype=np.int64)[:, None]
    keep = valid & (idx <= qpos) & (idx >= 0)
    s_flat = idx[keep]
    q_flat = np.broadcast_to(qpos, idx.shape)[keep]
    lin = s_flat * S + q_flat

    bo_rep = np.ascontiguousarray(np.broadcast_to(bo[None, :], (128, H)))

    # den row order in phase 3 is r = l*8 + ci for head h = 2*ci + l
    sel16 = np.zeros((NH, H), dtype=np.float32)
    ch = np.arange(H)
    sel16[((ch // HD) % 2) * 8 + ch // 128, ch] = 1.0
    sel7_host = np.zeros((HPC, 128), dtype=np.float32)
    for hh in range(HPC):
        sel7_host[hh, hh * HD : (hh + 1) * HD] = 1.0

    wq_scaled = Wq / np.sqrt(HD)

    # x pre-transposed: [st, p, hc, 512] with value x[st*512+s, hc*128+p]
    xT = np.ascontiguousarray(
        x2.T.reshape(8, 128, NQT, QT).transpose(2, 1, 0, 3)
    ).astype(ml_dtypes.bfloat16)

    def wslice(W, core):
        cs = slice(128 * core, 128 * (core + 1))
        return np.ascontiguousarray(
            W[:, cs].reshape(8, 128, 128).transpose(1, 0, 2)
        ).astype(ml_dtypes.bfloat16)

    wo_t = np.ascontiguousarray(Wo.reshape(8, 128, H).transpose(1, 0, 2)).astype(
        ml_dtypes.bfloat16
    )

    in_maps = []
    for core in range(NC):
        e_pack = np.empty((N_TILES, SC, HPC, QT), dtype=np.float16)
        for l in range(HPC):
            h = HPC * core + l
            w = np.exp(geo[h][keep].astype(np.float64))
            eT = np.bincount(lin, weights=w, minlength=S * S).reshape(S, S)
            for n, (t, c) in enumerate(TILE_LIST):
                e_pack[n, :, l, :] = eT[
                    c * SC : (c + 1) * SC, t * QT : (t + 1) * QT
                ].astype(np.float16)
        wo_own = np.ascontiguousarray(
            Wo[128 * core : 128 * (core + 1), :].reshape(HPC, HD, H).transpose(1, 0, 2)
        ).astype(ml_dtypes.bfloat16)
        in_maps.append(
            {
                "xT": xT,
                "wq": wslice(wq_scaled, core),
                "wk": wslice(Wk, core),
                "wv": wslice(Wv, core),
                "wo": wo_t,
                "bo_rep": bo_rep,
                "e_pack": e_pack,
                "sel16": sel16,
                "wo_own": wo_own,
                "sel7": sel7_host,
            }
        )
    return in_maps


LAST_RESULTS = None


def kernel(x, idx, valid, geo_bias, Wq, Wk, Wv, Wo, bo):
    global LAST_RESULTS
    b, s, h = np.asarray(x).shape
    assert (b, s, h) == (1, S, H)
    in_maps = _host_prep(x, idx, valid, geo_bias, Wq, Wk, Wv, Wo, bo)
    nc = _get_program()
    res = run_bass_kernel_spmd(nc, in_maps, core_ids=list(range(NC)))
    LAST_RESULTS = res
    y = np.empty((S, H), dtype=np.float32)
    y7 = np.zeros((QT, H), dtype=np.float32)
    for c in range(NC):
        yp = np.asarray(res.results[c]["y_part"], dtype=np.float32)
        y[384 * c : 384 * c + 384] = yp[0:384]
        y[3072 + 64 * c : 3072 + 64 * c + 64] = yp[384:448]
        y7 += np.asarray(res.results[c]["y7_part"], dtype=np.float32)
    y[3584:4096] = y7 + np.asarray(bo, dtype=np.float32)[None, :]
    return y.reshape(1, S, H).astype(np.float32)



# revision 75
# speedup vs baseline: 1.0775x; 1.0775x over previous
"""Sparse attention (per-query top-K) Trainium2 kernel, 8-core tensor-parallel.

Strategy (heads sharded 2-per-core, dense-score formulation):
  - Host folds idx/valid/geo_bias into per-(s,q) merged bias factors
    E[s,q] = sum_{j: idx[q,j]==s} exp(geo_bias[h,q,j]), stored as causal
    fp16 tiles.  This turns the per-query gather/softmax into dense math:
        A^T = E^T * exp(S^T - C),   S^T = K @ Q^T (feature-major)
        out^T = [V | 1]^T @ A^T     (row 64 = softmax denominator)
  - Single software-pipelined main loop: the projection matmuls for
    query-tile t+1 (bf16, x pre-transposed on host) are interleaved into
    the attention chunk loop of tile t so the PE fills the slack while
    ACT (the exp bottleneck) streams.  The two heads' 64-contraction
    score matmuls run concurrently in row-halves of the PE array; exp on
    ACT covers both heads per instruction; the E-multiply runs on DVE
    once per 4 chunks (fp16 2x mode); AV on PE in fp16.
  - AllToAll reshards the (unnormalized) head outputs + denominators so
    each core owns 512 query rows with all 16 heads, normalizes, and
    computes its o_proj row-slice with the full Wo in bf16.  Host
    concatenates.
"""

import sys

sys.path.insert(0, "/opt/trn_rl_repo")

import numpy as np
import ml_dtypes

from concourse import bacc, mybir, tile
from concourse.bass_utils import run_bass_kernel_spmd
from concourse.masks import make_identity

F32 = mybir.dt.float32
F16 = mybir.dt.float16
BF16 = mybir.dt.bfloat16

S = 4096
H = 1024
NH = 16
KSEL = 32
HD = 64
NC = 8
HPC = NH // NC  # 2 heads per core
QT = 512
NQT = S // QT
SC = 128
CSHIFT = 2.0
SLAB = 4  # s-chunks per E-tile DMA slab
QUAD = 2  # chunks per DVE multiply
AVLAG = 2  # quads the AV matmul trails the exp/multiply pipeline

TILE_LIST = [(t, c) for t in range(NQT) for c in range(4 * (t + 1))]
N_TILES = len(TILE_LIST)  # 144
TILE_IDX = {tc: n for n, tc in enumerate(TILE_LIST)}


def _build_program(phases=3, n_cores_build=NC):
    nc = bacc.Bacc(
        "TRN2", target_bir_lowering=False, debug=False, num_devices=n_cores_build
    )

    # host-pretransposed x: [st, p, hc, 512] (feature-major, bf16)
    xT_in = nc.dram_tensor("xT", [NQT, 128, 8, QT], BF16, kind="ExternalInput").ap()
    wq_in = nc.dram_tensor("wq", [128, 8, 128], BF16, kind="ExternalInput").ap()
    wk_in = nc.dram_tensor("wk", [128, 8, 128], BF16, kind="ExternalInput").ap()
    wv_in = nc.dram_tensor("wv", [128, 8, 128], BF16, kind="ExternalInput").ap()
    wo_in = nc.dram_tensor("wo", [128, 8, H], BF16, kind="ExternalInput").ap()
    bo_in = nc.dram_tensor("bo_rep", [128, H], F32, kind="ExternalInput").ap()
    e_in = nc.dram_tensor(
        "e_pack", [N_TILES, SC, HPC, QT], F16, kind="ExternalInput"
    ).ap()
    sel_in = nc.dram_tensor("sel16", [NH, H], F32, kind="ExternalInput").ap()
    wown_in = nc.dram_tensor(
        "wo_own", [HD, HPC, H], BF16, kind="ExternalInput"
    ).ap()
    sel7_in = nc.dram_tensor("sel7", [HPC, 128], F32, kind="ExternalInput").ap()
    y_out = nc.dram_tensor("y_part", [448, H], F16, kind="ExternalOutput").ap()
    # tile 7 is not resharded: each core emits its 2-head partial of the
    # full 512 rows; the host sums the 8 partials
    y7_out = nc.dram_tensor("y7_part", [QT, H], F16, kind="ExternalOutput").ap()

    with tile.TileContext(nc) as tc:
        with (
            tc.tile_pool(name="const", bufs=1) as constp,
            tc.tile_pool(name="persist", bufs=1) as persist,
            tc.tile_pool(name="dram", bufs=1, space="DRAM") as dram,
        ):
            ident = constp.tile([128, 128], F32, tag="ident")
            nbias = constp.tile([128, 1], F32, tag="nbias")

            wq_sb = constp.tile([128, 8, 128], BF16, tag="wq")
            wk_sb = constp.tile([128, 8, 128], BF16, tag="wk")
            wv_sb = constp.tile([128, 8, 128], BF16, tag="wv")
            # wo/bo/sel are loaded later (mid phase 2) to keep startup lean
            wo_sb = constp.tile([128, 8, H], BF16, tag="wo")
            bo_sb = constp.tile([128, H], F32, tag="bo")
            sel_sb = constp.tile([NH, H], F32, tag="sel")
            wown_sb = constp.tile([HD, HPC, H], BF16, tag="wown")
            # sel7[h, col] = 1 where col's head-dim block belongs to head h
            sel7 = constp.tile([HPC, 128], F32, tag="sel7")

            qT_sb = persist.tile([128, NQT, QT], BF16, tag="qT")
            kT_sb = persist.tile([128, NQT, QT], BF16, tag="kT")
            # wave-0/1 phase-3 inputs live in the persistent pool so their
            # loads + compute can be issued from inside the main loop
            den0_sb = persist.tile([NH, 384], F16, tag="den0")
            rden0_sb = persist.tile([NH, 384], F32, tag="rden0")
            oT0_sb = persist.tile([128, 8, 384], F16, tag="oT0")
            den1_sb = persist.tile([NH, 64], F16, tag="den1")
            rden1_sb = persist.tile([NH, 64], F32, tag="rden1")
            oT1_sb = persist.tile([128, 8, 64], F16, tag="oT1")
            v_sb = [
                persist.tile([128, S // SC, HD + 1], F16, tag=f"v{h}", name=f"v{h}")
                for h in range(HPC)
            ]


            # two AllToAll waves: wave 0 = q-tiles 0..5 (384 q/peer),
            # wave 1 = q-tile 6 (64 q/peer) — both resharded early enough
            # that their phase-3 interleaves into tile 7's chunk loop.
            # Tile 7 itself is never resharded: each core computes its
            # 2-head o_proj partial locally and the host sums them.
            WTILES = [list(range(0, 6)), [6]]
            WQ = [len(ws) * QT // NC for ws in WTILES]  # 384, 64
            NW = len(WTILES)
            WT0 = [ws[0] for ws in WTILES]  # first tile of each wave
            a2a_in = [
                dram.tile(
                    [NC, HPC * (HD + 1), WQ[w]], F16, tag=f"ain{w}", name=f"ain{w}"
                )
                for w in range(NW)
            ]
            a2a_out = [
                dram.tile(
                    [NC, HPC * (HD + 1), WQ[w]], F16, tag=f"aout{w}", name=f"aout{w}"
                )
                for w in range(NW)
            ]

            main_pools = (
                tc.tile_pool(name="xT", bufs=3),
                tc.tile_pool(name="vtmp", bufs=2),
                tc.tile_pool(name="zap", bufs=3),
                tc.tile_pool(name="ap", bufs=AVLAG + 2),
                tc.tile_pool(name="ep", bufs=7),
                tc.tile_pool(name="otp", bufs=2),
                tc.tile_pool(name="p3s", bufs=1),
                tc.tile_pool(name="p2s", bufs=2, space="PSUM"),
                tc.tile_pool(name="p2o", bufs=1, space="PSUM"),
                tc.tile_pool(name="p1ps", bufs=1, space="PSUM"),
            )
            xTp, vtmpp, zap, apool, epool, otp, p3s, p2s, p2o, p1ps = (
                p.__enter__() for p in main_pools
            )
            xT_tiles = {}

            # Queue discipline: E-slabs + xT alternate sync/gpsimd (the
            # two bulk queues); staging + y outputs ride the scalar queue
            # (their waits are always short, so the exp stream isn't
            # blocked); phase-3 reshard loads ride gpsimd where the only
            # long waits (on collectives) have dedicated slack.
            dma_no = [0]
            slab_no = [0]

            def big_dma(dst, src):
                if dma_no[0] % 2 == 0:
                    nc.sync.dma_start(dst, src)
                else:
                    nc.gpsimd.dma_start(dst, src)
                dma_no[0] += 1

            def load_xT(st, split=False):
                xt = xTp.tile([128, 8, QT], BF16, tag="xT", name=f"xT{st}")
                xT_tiles[st] = xt
                if split:  # per-hc chunks so the first matmul starts ASAP
                    for hc in range(8):
                        nc.sync.dma_start(xt[:, hc, :], xT_in[st, :, hc, :])
                    dma_no[0] += 1
                else:
                    big_dma(xt[:], xT_in[st])

            # ---- projection work for one s-tile, as two piece lists ----
            def proj_pieces(st):
                pieces = []
                state = {}

                def mk_mm(w_sb, crange, tag, fin):
                    def go():
                        if tag not in state:
                            state[tag] = p1ps.tile([128, QT], F32, tag="proj", name=f"proj_{tag}")
                        ps = state[tag]
                        for c in crange:
                            nc.tensor.matmul(
                                ps[:], w_sb[:, c, :], xT_tiles[st][:, c, :],
                                start=(c == 0), stop=(c == 7),
                            )
                        if fin is not None:
                            fin(ps)

                    return go

                def fin_q(ps):
                    nc.vector.tensor_copy(qT_sb[:, st, :], ps[:])

                def fin_k(ps):
                    nc.vector.tensor_copy(kT_sb[:, st, :], ps[:])

                def fin_v(ps):
                    vt = vtmpp.tile([128, QT], F32, tag="vt", name="vt")
                    state["vt"] = vt
                    nc.vector.tensor_copy(vt[:], ps[:])

                qk = [
                    mk_mm(wq_sb, range(0, 4), "q", None),
                    mk_mm(wq_sb, range(4, 8), "q", fin_q),
                    mk_mm(wk_sb, range(0, 4), "k", None),
                    mk_mm(wk_sb, range(4, 8), "k", fin_k),
                ]
                pieces.append(mk_mm(wv_sb, range(0, 4), "v", None))
                pieces.append(mk_mm(wv_sb, range(4, 8), "v", fin_v))

                def transpose_v():
                    vt = state["vt"]
                    ps_tv = p1ps.tile([128, QT], F32, tag="tp")
                    for i in range(4):
                        nc.tensor.transpose(
                            ps_tv[:, i * 128 : (i + 1) * 128],
                            vt[:, i * 128 : (i + 1) * 128],
                            ident[:],
                        )
                    ps_tv4 = ps_tv[:].rearrange("p (i h d) -> p i h d", i=4, h=HPC)
                    for h in range(HPC):
                        nc.vector.tensor_copy(
                            v_sb[h][:, st * 4 : (st + 1) * 4, 0:HD],
                            ps_tv4[:, :, h, :],
                        )

                pieces.append(transpose_v)
                return qk, pieces

            # ---- phase 3 (normalize + o_proj) for one wave, as pieces ----
            # y row ranges per wave in y_part: wave 0 -> 0:384,
            # wave 1 -> 384:448
            W0OFF = [0, 384]

            def p3_load(w, den_sb, oT_sb):
                # a2a_out[w] -> den/oT, on the gpsimd queue (gated on the
                # wave's collective; nothing else needs gpsimd then)
                for l in range(HPC):
                    nc.gpsimd.dma_start(
                        den_sb[l * 8 : (l + 1) * 8, :],
                        a2a_out[w][:, l * (HD + 1) + HD, :],
                    )
                for l in range(HPC):
                    nc.gpsimd.dma_start(
                        oT_sb[l * HD : (l + 1) * HD, :, :],
                        a2a_out[w][
                            :, l * (HD + 1) : l * (HD + 1) + HD, :
                        ].rearrange("c d q -> d c q"),
                    )

            def phase3_pieces(w, den_sb, rden_sb, oT_sb, sbpool, pspool, ypool):
                qw = WQ[w]
                pieces = []
                state = {}

                def recip():
                    nc.vector.reciprocal(rden_sb[:], den_sb[:])

                pieces.append(recip)

                def mk_norm(ci0):
                    def go():
                        if "on" not in state:
                            state["on"] = sbpool.tile(
                                [128, 8, qw], BF16, tag=f"on{w}", name=f"on{w}"
                            )
                        on_sb = state["on"]
                        for ci in range(ci0, ci0 + 2):
                            ps_b = pspool.tile([128, QT], F32, tag="proj", name="p3b")
                            nc.tensor.matmul(
                                ps_b[:, 0:qw],
                                sel_sb[:, ci * 128 : (ci + 1) * 128],
                                rden_sb[:],
                                start=True,
                                stop=True,
                            )
                            nc.vector.tensor_mul(
                                on_sb[:, ci, :], oT_sb[:, ci, :], ps_b[:, 0:qw]
                            )

                    return go

                for ci0 in range(0, 8, 2):
                    pieces.append(mk_norm(ci0))

                for qb0 in range(0, qw, 128):
                    bsz = min(128, qw - qb0)

                    def mk_oproj(qb0=qb0, bsz=bsz):
                        def go():
                            on_sb = state["on"]
                            y_sb = ypool.tile([128, H], F16, tag="y", name="p3y")
                            for nh2 in range(2):
                                ps_y = pspool.tile([128, QT], F32, tag="tp", name="p3yp")
                                for c in range(8):
                                    nc.tensor.matmul(
                                        ps_y[0:bsz, :],
                                        on_sb[:, c, qb0 : qb0 + bsz],
                                        wo_sb[:, c, nh2 * QT : (nh2 + 1) * QT],
                                        start=(c == 0),
                                        stop=(c == 7),
                                    )
                                nc.vector.tensor_add(
                                    y_sb[0:bsz, nh2 * QT : (nh2 + 1) * QT],
                                    ps_y[0:bsz, :],
                                    bo_sb[0:bsz, nh2 * QT : (nh2 + 1) * QT],
                                )
                            nc.scalar.dma_start(
                                y_out[
                                    W0OFF[w] + qb0 : W0OFF[w] + qb0 + bsz, :
                                ],
                                y_sb[0:bsz, :],
                            )

                        return go

                    pieces.append(mk_oproj())
                return pieces

            # ---- main software-pipelined loop ---------------------------
            # first-needed bytes on separate queues so nothing serializes:
            # xT0 streams on sync (feeds the first matmuls), weights + xT1
            # go on gpsimd, and tile 0/1's E-slabs land right behind xT0
            # on sync (they gate the first DVE multiply)
            xt0 = xTp.tile([128, 8, QT], BF16, tag="xT", name="xT0")
            xT_tiles[0] = xt0
            for hc in range(8):
                nc.sync.dma_start(xt0[:, hc, :], xT_in[0, :, hc, :])
            nc.gpsimd.dma_start(wq_sb[:], wq_in[:])
            nc.gpsimd.dma_start(wk_sb[:], wk_in[:])
            nc.gpsimd.dma_start(wv_sb[:], wv_in[:])
            xt1 = xTp.tile([128, 8, QT], BF16, tag="xT", name="xT1")
            xT_tiles[1] = xt1
            nc.gpsimd.dma_start(xt1[:], xT_in[1])
            # constants AFTER the preamble DMA triggers: these gpsimd ops
            # are slow and must not delay the weight/x loads
            make_identity(nc, ident[:])
            nc.gpsimd.memset(nbias[:], -CSHIFT)
            # prime the exp table-set while the first DMAs stream
            warm = constp.tile([1, 1], F32, tag="warm")
            nc.scalar.activation(
                warm[:], nbias[0:1, :], mybir.ActivationFunctionType.Exp
            )
            for h in range(HPC):
                # only the ones-column matters; cols 0:64 are overwritten
                nc.gpsimd.memset(v_sb[h][:, :, HD : HD + 1], 1.0)
            # run proj(0) + qk(1) inline up front (PE is DMA-bound then);
            # thereafter tile t's loop carries v(t+1) + qk(t+2), so tiles
            # 6-7 are free for the phase-3 / epilogue work
            qk0, v0 = proj_pieces(0)
            for p in qk0:
                p()
            for p in v0:
                p()
            qk1, v1 = proj_pieces(1)
            for p in qk1:
                p()
            carry_v = v1
            qk_next, v_next = proj_pieces(2)

            for t in range(NQT):
                nchunks = 4 * (t + 1)
                nquads = nchunks // QUAD
                # stream this tile's E slabs (demand-gated by pool bufs)
                slabs = []
                for g0 in range(0, nchunks, SLAB):
                    gsz = min(SLAB, nchunks - g0)
                    e_slab = epool.tile(
                        [128, SLAB, HPC, QT], F16, tag="e", name="e_slab"
                    )
                    n0 = TILE_IDX[(t, g0)]
                    big_dma(
                        e_slab[:, 0:gsz, :, :],
                        e_in[n0 : n0 + gsz].rearrange("n p h q -> p n h q"),
                    )
                    slabs.append(e_slab)
                if t == 7:
                    # wave-0 reshard results (collective fired at t=5);
                    # emitted after the slab triggers so gpsimd's slabs
                    # aren't stuck behind the collective-gated wait
                    p3_load(0, den0_sb, oT0_sb)
                if t + 2 < NQT:
                    load_xT(t + 2)
                if t == 4:
                    nc.gpsimd.dma_start(wo_sb[:], wo_in[:])
                    nc.gpsimd.dma_start(bo_sb[:], bo_in[:])
                    nc.gpsimd.dma_start(sel_sb[:], sel_in[:])
                    nc.gpsimd.dma_start(wown_sb[:], wown_in[:])
                    nc.gpsimd.dma_start(sel7[:], sel7_in[:])

                # pieces to interleave: tile t+1's V projection, then
                # tile t+2's Q/K projection; at t == 7 only the phase-3
                # work of waves 0/1 (their collectives fired at t = 5/6)
                pieces = carry_v + qk_next
                if t + 2 < NQT:
                    carry_v = v_next
                    if t + 3 < NQT:
                        qk_next, v_next = proj_pieces(t + 3)
                    else:
                        qk_next = []
                else:
                    carry_v = []
                    qk_next = []
                qt_list = None
                if t == 7:
                    # waves 0/1 phase 3 interleave into tile 7's second
                    # half (wave-0's collective is in flight until
                    # ~mid-tile, wave-1's a bit longer)
                    p3w0 = phase3_pieces(0, den0_sb, rden0_sb, oT0_sb, p3s, p1ps, otp)
                    p3w1 = phase3_pieces(1, den1_sb, rden1_sb, oT1_sb, p3s, p1ps, otp)
                    n0 = len(pieces)
                    pieces = (
                        pieces + p3w0 + [lambda: p3_load(1, den1_sb, oT1_sb)] + p3w1
                    )
                    n2 = n0 + len(p3w0) + 1
                    qt_list = []
                    for qd in range(16):
                        if qd < 8:
                            qt_list.append(n0 + -(-(qd + 1) * len(p3w0) // 8))
                        elif qd < 11:
                            qt_list.append(n0 + len(p3w0))
                        else:
                            qt_list.append(n2 + -(-(qd - 10) * len(p3w1) // 5))
                npc = [0]

                def emit_proj_piece(n=1):
                    for _ in range(n):
                        if npc[0] < len(pieces):
                            pieces[npc[0]]()
                            npc[0] += 1

                ps_o = [
                    p2o.tile([HD + 1, QT], F32, tag=f"po{h}", name=f"po{h}")
                    for h in range(HPC)
                ]

                def emit_st_exp(t_, c, z_view, qlo=0):
                    ps_s2 = p2s.tile([128, 2 * QT], F32, tag="ps2", name="ps2")
                    for h in range(HPC):
                        nc.tensor.matmul(
                            ps_s2[:, h * QT + qlo : (h + 1) * QT],
                            kT_sb[
                                h * HD : (h + 1) * HD,
                                c // 4,
                                (c % 4) * 128 : (c % 4 + 1) * 128,
                            ],
                            qT_sb[h * HD : (h + 1) * HD, t_, qlo:],
                            start=True,
                            stop=True,
                        )
                    nc.scalar.activation(
                        z_view[:, :, qlo:],
                        ps_s2[:].rearrange("p (h q) -> p h q", h=HPC)[:, :, qlo:],
                        mybir.ActivationFunctionType.Exp,
                        bias=nbias[:],
                    )

                def chunk_qlo(c):
                    # columns below the causal diagonal of this s-chunk
                    # have E == 0; skip them
                    return max(0, 128 * (c - (nchunks - 4)))

                def emit_av(pc0, pa, last):
                    for pc in range(QUAD):
                        c = pc0 + pc
                        qlo = chunk_qlo(c)
                        for h in range(HPC):
                            nc.tensor.matmul(
                                ps_o[h][:, qlo:],
                                v_sb[h][:, c, :],
                                pa[:, pc, h, qlo:],
                                start=(c == 0),
                                stop=(last and c == nchunks - 1),
                            )

                # AV matmuls lag AVLAG quads behind the mul so the serial
                # scores->exp->mul->AV chain spreads over several slots
                pending_av = []
                for qd in range(nquads):
                    c0 = qd * QUAD
                    e_slab = slabs[c0 // SLAB]
                    z_sb = zap.tile([128, QUAD, HPC, QT], F16, tag="z", name="z4")
                    a_sb = apool.tile([128, QUAD, HPC, QT], F16, tag="a", name="a4")
                    for pc in range(QUAD):
                        emit_st_exp(
                            t, c0 + pc, z_sb[:, pc, :, :], chunk_qlo(c0 + pc)
                        )
                    c_loc = c0 % SLAB
                    # trimming the pair multiply to the smaller offset is
                    # safe: E is zero below the diagonal, so the extra
                    # columns multiply to exactly 0
                    qlo = chunk_qlo(c0)
                    nc.vector.tensor_mul(
                        a_sb[:, :, :, qlo:],
                        z_sb[:, :, :, qlo:],
                        e_slab[:, c_loc : c_loc + QUAD, :, qlo:],
                    )
                    if qt_list is not None:
                        emit_proj_piece(qt_list[qd] - npc[0])
                    else:
                        emit_proj_piece(
                            -(-(len(pieces) - npc[0]) // max(1, nquads - qd))
                            if pieces
                            else 0
                        )
                    pending_av.append((c0, a_sb))
                    if len(pending_av) > AVLAG:
                        pc0, pa = pending_av.pop(0)
                        emit_av(pc0, pa, False)
                emit_proj_piece(len(pieces))
                while pending_av:
                    pc0, pa = pending_av.pop(0)
                    emit_av(pc0, pa, not pending_av)

                if t < 7:
                    ot_sb = otp.tile([HD + 1, HPC, QT], F16, tag="ot")
                    for h in range(HPC):
                        nc.vector.tensor_copy(ot_sb[:, h, :], ps_o[h][:])
                    # stage this tile's columns into the owning peers' slots
                    w = 0 if t < 6 else 1
                    wq0 = WQ[w]
                    gcol0 = (t - WT0[w]) * QT
                    for p in range(NC):
                        lo = max(gcol0, wq0 * p)
                        hi = min(gcol0 + QT, wq0 * p + wq0)
                        if lo < hi:
                            nc.gpsimd.dma_start(
                                a2a_in[w][p][
                                    :, lo - wq0 * p : hi - wq0 * p
                                ].rearrange("(h pp) q -> pp h q", h=HPC),
                                ot_sb[:, :, lo - gcol0 : hi - gcol0],
                            )
                    if phases >= 2.5 and t in (5, 6):
                        w = t - 5
                        nc.gpsimd.collective_compute(
                            "AllToAll",
                            mybir.AluOpType.bypass,
                            replica_groups=[list(range(NC))],
                            ins=[a2a_in[w].opt()],
                            outs=[a2a_out[w].opt()],
                        )
                else:
                    # tile 7 local epilogue: normalize own 2 heads and emit
                    # this core's o_proj partial; the host sums the 8
                    # partials (no collective on the critical tail)
                    ot_sb = otp.tile([HD + 1, HPC, QT], F16, tag="ot")
                    for h in range(HPC):
                        nc.vector.tensor_copy(ot_sb[:, h, :], ps_o[h][:])
                    den7 = p3s.tile([HPC, QT], F16, tag="den7")
                    for h in range(HPC):
                        # partition shift (64 -> h) must go through DMA
                        nc.gpsimd.dma_start(
                            den7[h : h + 1, :], ot_sb[HD : HD + 1, h, :]
                        )
                    rden7 = p3s.tile([HPC, QT], F32, tag="rden7")
                    nc.vector.reciprocal(rden7[:], den7[:])
                    on7 = [
                        p3s.tile([HD, QT], BF16, tag=f"on7_{h}", name=f"on7_{h}")
                        for h in range(HPC)
                    ]
                    for h in range(HPC):
                        ps_b7 = p1ps.tile(
                            [128, QT], F32, tag="proj", name=f"bc7_{h}"
                        )
                        nc.tensor.matmul(
                            ps_b7[0:HD, :],
                            sel7[:, h * HD : (h + 1) * HD],
                            rden7[:],
                            start=True,
                            stop=True,
                        )
                        nc.vector.tensor_mul(
                            on7[h][:], ot_sb[0:HD, h, :], ps_b7[0:HD, :]
                        )
                    for qb in range(4):
                        ps_y7 = p2s.tile(
                            [128, 2 * QT], F32, tag="ps2", name="y7ps"
                        )
                        for nh2 in range(2):
                            for h in range(HPC):
                                nc.tensor.matmul(
                                    ps_y7[:, nh2 * QT : (nh2 + 1) * QT],
                                    on7[h][:, qb * 128 : (qb + 1) * 128],
                                    wown_sb[:, h, nh2 * QT : (nh2 + 1) * QT],
                                    start=(h == 0),
                                    stop=(h == HPC - 1),
                                )
                        y7_sb = otp.tile([128, H], F16, tag="y7", name="y7")
                        nc.vector.tensor_copy(y7_sb[:], ps_y7[:])
                        nc.scalar.dma_start(
                            y7_out[qb * 128 : (qb + 1) * 128, :], y7_sb[:]
                        )

            for p in reversed(main_pools):
                p.__exit__(None, None, None)

            if phases < 3:
                with tc.tile_pool(name="dbg", bufs=1) as dbgp:
                    dbg = dbgp.tile([128, 2, QT], F32, tag="dbg")
                    nc.vector.tensor_copy(dbg[:], qT_sb[:, 0:2, :].bitcast(F32))
                    nc.sync.dma_start(
                        y_out[0:128, :], dbg[:].rearrange("p a b -> p (a b)")
                    )

    nc.compile()
    return nc


_PROGRAM_CACHE = {}


def _get_program():
    if "nc" not in _PROGRAM_CACHE:
        _PROGRAM_CACHE["nc"] = _build_program()
    return _PROGRAM_CACHE["nc"]


def _host_prep(x, idx, valid, geo_bias, Wq, Wk, Wv, Wo, bo):
    x2 = np.ascontiguousarray(np.asarray(x, dtype=np.float32).reshape(S, H))
    idx = np.asarray(idx).astype(np.int64)
    valid = np.asarray(valid).astype(bool)
    geo = np.asarray(geo_bias, dtype=np.float32)
    Wq = np.asarray(Wq, dtype=np.float32)
    Wk = np.asarray(Wk, dtype=np.float32)
    Wv = np.asarray(Wv, dtype=np.float32)
    Wo = np.asarray(Wo, dtype=np.float32)
    bo = np.asarray(bo, dtype=np.float32)

    qpos = np.arange(S, dtype=np.int64)[:, None]
    keep = valid & (idx <= qpos) & (idx >= 0)
    s_flat = idx[keep]
    q_flat = np.broadcast_to(qpos, idx.shape)[keep]
    lin = s_flat * S + q_flat

    bo_rep = np.ascontiguousarray(np.broadcast_to(bo[None, :], (128, H)))

    # den row order in phase 3 is r = l*8 + ci for head h = 2*ci + l
    sel16 = np.zeros((NH, H), dtype=np.float32)
    ch = np.arange(H)
    sel16[((ch // HD) % 2) * 8 + ch // 128, ch] = 1.0
    sel7_host = np.zeros((HPC, 128), dtype=np.float32)
    for hh in range(HPC):
        sel7_host[hh, hh * HD : (hh + 1) * HD] = 1.0

    wq_scaled = Wq / np.sqrt(HD)

    # x pre-transposed: [st, p, hc, 512] with value x[st*512+s, hc*128+p]
    xT = np.ascontiguousarray(
        x2.T.reshape(8, 128, NQT, QT).transpose(2, 1, 0, 3)
    ).astype(ml_dtypes.bfloat16)

    def wslice(W, core):
        cs = slice(128 * core, 128 * (core + 1))
        return np.ascontiguousarray(
            W[:, cs].reshape(8, 128, 128).transpose(1, 0, 2)
        ).astype(ml_dtypes.bfloat16)

    wo_t = np.ascontiguousarray(Wo.reshape(8, 128, H).transpose(1, 0, 2)).astype(
        ml_dtypes.bfloat16
    )

    in_maps = []
    for core in range(NC):
        e_pack = np.empty((N_TILES, SC, HPC, QT), dtype=np.float16)
        for l in range(HPC):
            h = HPC * core + l
            w = np.exp(geo[h][keep].astype(np.float64))
            eT = np.bincount(lin, weights=w, minlength=S * S).reshape(S, S)
            for n, (t, c) in enumerate(TILE_LIST):
                e_pack[n, :, l, :] = eT[
                    c * SC : (c + 1) * SC, t * QT : (t + 1) * QT
                ].astype(np.float16)
        wo_own = np.ascontiguousarray(
            Wo[128 * core : 128 * (core + 1), :].reshape(HPC, HD, H).transpose(1, 0, 2)
        ).astype(ml_dtypes.bfloat16)
        in_maps.append(
            {
                "xT": xT,
                "wq": wslice(wq_scaled, core),
                "wk": wslice(Wk, core),
                "wv": wslice(Wv, core),
                "wo": wo_t,
                "bo_rep": bo_rep,
                "e_pack": e_pack,
                "sel16": sel16,
                "wo_own": wo_own,
                "sel7": sel7_host,
            }
        )
    return in_maps


LAST_RESULTS = None


def kernel(x, idx, valid, geo_bias, Wq, Wk, Wv, Wo, bo):
    global LAST_RESULTS
    b, s, h = np.asarray(x).shape
    assert (b, s, h) == (1, S, H)
    in_maps = _host_prep(x, idx, valid, geo_bias, Wq, Wk, Wv, Wo, bo)
    nc = _get_program()
    res = run_bass_kernel_spmd(nc, in_maps, core_ids=list(range(NC)))
    LAST_RESULTS = res
    y = np.empty((S, H), dtype=np.float32)
    y7 = np.zeros((QT, H), dtype=np.float32)
    for c in range(NC):
        yp = np.asarray(res.results[c]["y_part"], dtype=np.float32)
        y[384 * c : 384 * c + 384] = yp[0:384]
        y[3072 + 64 * c : 3072 + 64 * c + 64] = yp[384:448]
        y7 += np.asarray(res.results[c]["y7_part"], dtype=np.float32)
    y[3584:4096] = y7 + np.asarray(bo, dtype=np.float32)[None, :]
    return y.reshape(1, S, H).astype(np.float32)

